# revision 3
# baseline (speedup 1.0000x reference)
"""Channel-attention (nn_CAttention) Trainium2 kernel, v2.

Full inputs in, full output out. Data-parallel over batch B=8 across 8
NeuronCores (one batch element per core); the small [C,C] projection weight
is replicated (passed pre-transposed, cast to bf16, as weight prep).

Per-core math (b fixed, head n in [0,8), c=256 channels, s=2048 spatial):
  qh, kh, vh = q[b].reshape(8, 256, 2048) etc (contiguous view)
  qn = qh / ||qh||_row ; kn likewise          (l2 norm along s)
  GT[d, c] = sum_s kn[d,s] qn[c,s]            (= attn^T)
  sig = sigmoid(GT)
  out_h[c, s] = sum_d sig[d, c] vh[d, s]
  X[32n+j, q*2048+s] = out_h[8j+q, s]         (head -> original channel layout)
  O = W @ X                                   (1x1 conv projection)

v2 vs v1: the out-matmuls are full-width [128, 512] (lhsT = sig columns,
rhs = v) instead of 32-row column-packed tiles — 4x less PE time on that
phase — and the head->channel permutation is done by SBUF->SBUF DMA with a
partition-split AP (partition 8j+q -> partition 16ct+j at free offset
q*2048), which streams under PE work. Projection reads the permuted X and
chases per 512-column chunk; PSUM->SBUF copies alternate DVE/Act.

v3 vs v2: norms move off the critical path — computed one head ahead, as
fused multiply-reduce on DVE (one [128,4] sqrt / reciprocal per head), with
only q's norm folded into the transpose diagonal; k's norm is applied as
the sigmoid's per-partition input scale. tsb copies split DVE/Act 6/10 so
the s-block loop is PE-limited. Output is stored bf16 (host converts back
to f32) to cut store traffic in half.
"""

import os

os.environ.setdefault("JAX_PLATFORMS", "axon,cpu")

import numpy as np
import ml_dtypes
from contextlib import ExitStack

import concourse.bass as bass
import concourse.tile as tile
from concourse import mybir
from concourse._compat import with_exitstack
from concourse.bass import ts, ds
from concourse.bass_utils import run_bass_kernel_spmd
from concourse.masks import make_identity
from concourse.vector_clock import ScopedClock

B, C, HH, WW = 8, 256, 128, 128
NH = 8
S = (HH * WW) // NH  # 2048
HW = HH * WW  # 16384
EPS = 1e-12

F32 = mybir.dt.float32
BF16 = mybir.dt.bfloat16
AF = mybir.ActivationFunctionType

_MAX_DRAIN_WAITS = 1


def _install_drain_patch():
    """This walrus build rejects >1 sync wait on a CTRL instruction; spread
    the TileContext final-drain waits across chained wait-nops on SP."""

    def _drain_and_barrier_split(self, tick_clock, wait_clock):
        nc = self.nc
        drain_inst = nc.sync.drain()
        wait_clock.add_sem_waits(
            drain_inst.ins, ScopedClock({None: tick_clock.global_clock})
        )
        si = drain_inst.ins.sync_info
        waits = list(si.on_wait) if si is not None else []
        if len(waits) > _MAX_DRAIN_WAITS:
            drain_inst.ins.sync_info = mybir.SyncInfo(
                on_wait=waits[:_MAX_DRAIN_WAITS], on_update=[]
            )
            for i in range(_MAX_DRAIN_WAITS, len(waits), _MAX_DRAIN_WAITS):
                nop = nc.sync.nop(nofuse=True, hint="drain_wait_split")
                nop.ins.sync_info = mybir.SyncInfo(
                    on_wait=waits[i : i + _MAX_DRAIN_WAITS], on_update=[]
                )
        nc.all_engine_barrier()
        assert self.sems is not None
        popped = nc._tile_sem_poison_stack.pop()
        assert popped is self._sem_poison
        nc.clear_and_free_semaphores(list(self.sems.allocated().values()))
        nc.all_engine_barrier()

    tile.TileContext._drain_and_barrier = _drain_and_barrier_split


def _split_excess_waits(nc, max_waits=_MAX_DRAIN_WAITS):
    """This walrus build allows only one sync-wait command per instruction;
    hoist extra waits into nofuse NOPs on the same engine just before."""
    n_split = 0
    for f in nc.m.functions:
        for blk in f.blocks:
            il = blk.instructions
            new = []
            for inst in il:
                si = inst.sync_info
                waits = list(si.on_wait) if si is not None else []
                if len(waits) > max_waits:
                    extra, keep = waits[:-max_waits], waits[-max_waits:]
                    for j in range(0, len(extra), max_waits):
                        nop = mybir.InstNoOp(
                            name=f"{inst.name}-wsplit{j}",
                            sync_info=mybir.SyncInfo(
                                on_wait=extra[j : j + max_waits], on_update=[]
                            ),
                            bass_nofuse=True,
                            engine=inst.engine,
                        )
                        new.append(nop)
                    inst.sync_info = mybir.SyncInfo(
                        on_wait=keep, on_update=list(si.on_update)
                    )
                    n_split += 1
                new.append(inst)
            if len(new) != len(il):
                il[:] = new
    return n_split


def _cattn_consts(ctx: ExitStack, tc: tile.TileContext, wt):
    """One-time constants: bf16 identity matrix and the transposed projection
    weight (bf16) resident in SBUF."""
    nc = tc.nc
    consts = ctx.enter_context(tc.tile_pool(name="consts", bufs=1))
    ident = consts.tile([128, 128], BF16)
    make_identity(nc, ident)
    wt_sb = consts.tile([128, 2, 256], BF16)
    nc.sync.dma_start(out=wt_sb, in_=wt[:].rearrange("(ch p) o -> p ch o", p=128))
    return ident, wt_sb


@with_exitstack
def _cattn_body(ctx: ExitStack, tc: tile.TileContext, q, k, v, ident, wt_sb, out):
    nc = tc.nc

    natp = ctx.enter_context(tc.tile_pool(name="nat", bufs=4))
    vp = ctx.enter_context(tc.tile_pool(name="v", bufs=2))
    sqs = ctx.enter_context(tc.tile_pool(name="sqs", bufs=2))
    stat = ctx.enter_context(tc.tile_pool(name="stat", bufs=8))
    dpool = ctx.enter_context(tc.tile_pool(name="diag", bufs=4))
    qkt = ctx.enter_context(tc.tile_pool(name="qkt", bufs=3))
    sgp = ctx.enter_context(tc.tile_pool(name="sg", bufs=2))
    ohp = ctx.enter_context(tc.tile_pool(name="oh", bufs=3))
    xp = ctx.enter_context(tc.tile_pool(name="x", bufs=1))
    obuf = ctx.enter_context(tc.tile_pool(name="obuf", bufs=3))
    # PSUM: 2x[128,1024] transpose + 2x[128,512] GT/proj + 2x[128,512] out
    tpsum = ctx.enter_context(tc.tile_pool(name="tpsum", bufs=2, space="PSUM"))
    gpsum = ctx.enter_context(tc.tile_pool(name="gpsum", bufs=2, space="PSUM"))
    bpsum = ctx.enter_context(tc.tile_pool(name="bpsum", bufs=2, space="PSUM"))

    X = xp.tile([128, 2, HW], BF16)

    qk_loads = [None] * NH

    def ensure_qk(n):
        if n >= NH or qk_loads[n] is not None:
            return
        pair = {}
        for name, src in (("q", q), ("k", k)):
            nat = natp.tile([128, 2, S], BF16, tag=f"{name}nat")
            nc.gpsimd.dma_start(
                out=nat, in_=src[n].rearrange("(a p) s -> p a s", p=128)
            )
            pair[name] = nat
        qk_loads[n] = pair

    def load_v(n):
        vt = vp.tile([128, 2, S], BF16, tag="v")
        nc.gpsimd.dma_start(
            out=vt, in_=v[n].rearrange("(a p) s -> p a s", p=128)
        )
        return vt

    hstate = [dict() for _ in range(NH)]

    def sq_step(n, i):
        """Square+accum for one (tensor, ct) of head n on Act."""
        name, ct = (("q", 0), ("q", 1), ("k", 0), ("k", 1))[i]

        def emit():
            st = hstate[n]
            if "ssq4" not in st:
                st["ssq4"] = stat.tile(
                    [128, 4], F32, tag="ssq4", name=f"ssq4_{n}"
                )
            scr = sqs.tile([128, S], BF16, tag="scr")
            nc.scalar.activation(
                out=scr,
                in_=qk_loads[n][name][:, ct],
                func=AF.Square,
                accum_out=st["ssq4"][:, i : i + 1],
            )

        return emit

    def finish_q(n):
        """sqrt + reciprocal + diag tiles for q's norm (head n)."""

        def emit():
            st = hstate[n]
            nrm = stat.tile([128, 2], F32, tag="nrmq")
            nc.scalar.activation(out=nrm, in_=st["ssq4"][:, 0:2], func=AF.Sqrt)
            rr = stat.tile([128, 2], F32, tag="rrq")
            nc.vector.reciprocal(out=rr, in_=nrm)
            st["Dq"] = []
            for ct in range(2):
                D = dpool.tile([128, 128], BF16, tag="D")
                nc.vector.tensor_scalar_mul(
                    out=D, in0=ident, scalar1=rr[:, ct : ct + 1]
                )
                st["Dq"].append(D)

        return emit

    def finish_k(n):
        """sqrt + reciprocal for k's norm (the sigmoid's input scale)."""

        def emit():
            st = hstate[n]
            nrm = stat.tile([128, 2], F32, tag="nrmk")
            nc.scalar.activation(out=nrm, in_=st["ssq4"][:, 2:4], func=AF.Sqrt)
            rrk = stat.tile([128, 2], F32, tag="rrk")
            nc.vector.reciprocal(out=rrk, in_=nrm)
            st["rrk"] = rrk

        return emit

    def sigmoid_step(n):
        def emit():
            st = hstate[n]
            sg = sgp.tile([128, 2, 256], BF16, tag="sg")
            for dt_ in range(2):
                nc.scalar.activation(
                    out=sg[:, dt_],
                    in_=st["gps"][dt_][:, 0:256],
                    func=AF.Sigmoid,
                    scale=st["rrk"][:, dt_ : dt_ + 1],
                )
            st["sg"] = sg

        return emit

    def out_chunk_step(n, idx):
        """One [128,512] out-matmul chunk (ct=idx//4, sc=idx%4) + copy."""
        ct, sc = idx // 4, idx % 4

        def emit():
            st = hstate[n]
            if "oh" not in st:
                st["oh"] = ohp.tile([128, 2, S], BF16, tag="oh", name=f"oh_{n}")
            ps = bpsum.tile([128, 512], F32, tag="ops")
            for db in range(2):
                nc.tensor.matmul(
                    ps,
                    lhsT=st["sg"][:, db, ts(ct, 128)],
                    rhs=st["vt"][:, db, ts(sc, 512)],
                    start=(db == 0),
                    stop=(db == 1),
                )
            if idx in (0, 4):
                nc.scalar.copy(out=st["oh"][:, ct, ts(sc, 512)], in_=ps)
            else:
                nc.vector.tensor_copy(out=st["oh"][:, ct, ts(sc, 512)], in_=ps)

        return emit

    def permute_step(n, ct):
        """Permute-DMA half of out_h into X. The read side stays the plain
        [128, 2048] AP (full-tile coverage for dependency tracking); the
        write side's flat iteration order (j, q, s) pairs element p*2048+s
        with (j=p//8, q=p%8, s) — exactly the head->channel permutation."""

        def emit():
            st = hstate[n]
            dst = X[ds(32 * (n % 4) + 16 * ct, 16), n // 4].rearrange(
                "j (q s) -> j q s", q=8
            )
            nc.gpsimd.dma_start(out=dst, in_=st["oh"][:, ct])

        return emit

    def tail_steps(n):
        """Post-GT work of head n, interleaved into head n+1's pair loop."""
        return [
            sigmoid_step(n),
            out_chunk_step(n, 0),
            out_chunk_step(n, 1),
            out_chunk_step(n, 2),
            out_chunk_step(n, 3),
            out_chunk_step(n, 4),
            permute_step(n, 0),
            out_chunk_step(n, 5),
            out_chunk_step(n, 6),
            out_chunk_step(n, 7),
            permute_step(n, 1),
        ]

    def norm_steps(n):
        """Norm work of head n, interleaved into head n-1's pair loop."""
        return [
            sq_step(n, 0),
            sq_step(n, 1),
            finish_q(n),
            sq_step(n, 2),
            sq_step(n, 3),
            finish_k(n),
        ]

    def process_head(n, fillers):
        """Fused transpose (+q-norm) -> GT accumulation.

        Transposes run in s-block pairs into a [128,1024] PSUM tile with one
        merged PSUM->SBUF copy per pair; GT matmuls for pair p are emitted
        after the transposes of pair p+1 so PE never waits on the copy of
        the block it is about to contract. `fillers` (the previous head's
        sigmoid/out/permute and the next head's norms) are drained two per
        pair so every engine stays fed throughout the loop.
        """
        st = hstate[n]
        pair = qk_loads[n]
        qk_loads[n] = None
        # GT accumulators: one PSUM bank per dt chain (accumulation groups
        # may not interleave within a bank)
        st["gps"] = [
            gpsum.tile([128, 512], F32, tag="gps", name=f"gps{i}")
            for i in range(2)
        ]
        tsbs = [None] * 8
        fillers = list(fillers)

        def emit_gt(p):
            for half in range(2):
                sb = 2 * p + half
                for dt_ in range(2):
                    nc.tensor.matmul(
                        st["gps"][dt_][:, 0:256],
                        lhsT=tsbs[p][:, ds(half * 512 + 256 + dt_ * 128, 128)],
                        rhs=tsbs[p][:, ds(half * 512, 256)],
                        start=(sb == 0),
                        stop=(sb == 15),
                    )
            tsbs[p] = None

        Dq = st["Dq"]
        for p in range(8):
            ps = tpsum.tile([128, 1024], F32, tag="tps")
            for half in range(2):
                sb = 2 * p + half
                for ti, name in enumerate(("q", "k")):
                    nat = pair[name]
                    for ct in range(2):
                        nc.tensor.matmul(
                            ps[
                                :,
                                ds(half * 512 + ti * 256 + ct * 128, 128),
                            ],
                            lhsT=nat[:, ct, ts(sb, 128)],
                            rhs=Dq[ct] if name == "q" else ident,
                            start=True,
                            stop=True,
                        )
            tsb = qkt.tile([128, 1024], BF16, tag="tsb")
            if p % 4 == 3:
                nc.scalar.copy(out=tsb, in_=ps)
            else:
                nc.vector.tensor_copy(out=tsb, in_=ps)
            tsbs[p] = tsb
            for _ in range(2):
                if fillers:
                    fillers.pop(0)()
            if p > 0:
                emit_gt(p - 1)
        while fillers:
            fillers.pop(0)()
        emit_gt(7)

    def proj_chunk(tc_):
        """Projection + bf16 output stream for one 512-column t-range."""
        t0 = tc_ * 512
        ob = obuf.tile([128, 2, 512], BF16, tag="ob")
        for ot in range(2):
            pps = bpsum.tile([128, 512], F32, tag="ops")
            for ch in range(2):
                nc.tensor.matmul(
                    pps,
                    lhsT=wt_sb[:, ch, ts(ot, 128)],
                    rhs=X[:, ch, ds(t0, 512)],
                    start=(ch == 0),
                    stop=(ch == 1),
                )
            if ot == 0:
                nc.vector.tensor_copy(out=ob[:, ot], in_=pps)
            else:
                nc.scalar.copy(out=ob[:, ot], in_=pps)
        eng = nc.sync if tc_ % 2 == 0 else nc.gpsimd
        eng.dma_start(
            out=out.rearrange("(o2 p) t -> p o2 t", p=128)[:, :, ds(t0, 512)],
            in_=ob,
        )

    ensure_qk(0)
    ensure_qk(1)
    ensure_qk(2)
    # head 0's norms run up front (q first so the pair loop can start
    # before k's norm resolves)
    for s in norm_steps(0):
        s()
    for n in range(NH):
        ensure_qk(n + 2)
        ensure_qk(n + 3)
        hstate[n]["vt"] = load_v(n)
        fillers = []
        if n > 0:
            fillers.extend(tail_steps(n - 1))
        if n + 1 < NH:
            fillers.extend(norm_steps(n + 1))
        # interleave: previous head's tail and next head's norms alternate
        # so Act (sigmoid/squares) and DVE (copies) both stay fed
        if n > 0 and n + 1 < NH:
            t_, m_ = tail_steps(n - 1), norm_steps(n + 1)
            fillers = [t_[0]]  # sigmoid first
            ti, mi = 1, 0
            while ti < len(t_) or mi < len(m_):
                if mi < len(m_):
                    fillers.append(m_[mi])
                    mi += 1
                if ti < len(t_):
                    fillers.append(t_[ti])
                    ti += 1
                if ti < len(t_):
                    fillers.append(t_[ti])
                    ti += 1
        process_head(n, fillers)

    # head 7's tail, then the projection sweep
    for s in tail_steps(NH - 1):
        s()
    for tc_ in range(32):
        proj_chunk(tc_)


_NC_CACHE = {}


def _build_nc(repeats=1):
    if repeats in _NC_CACHE:
        return _NC_CACHE[repeats]
    no_walrus = bool(os.environ.get("NO_WALRUS_PATCH"))
    if not no_walrus:
        _install_drain_patch()
    nc = bass.Bass(num_swdge_queues=4)
    q = nc.declare_dram_parameter("q", [NH, C, S], F32, isOutput=False)
    k = nc.declare_dram_parameter("k", [NH, C, S], F32, isOutput=False)
    v = nc.declare_dram_parameter("v", [NH, C, S], F32, isOutput=False)
    wt = nc.declare_dram_parameter("wt", [C, C], BF16, isOutput=False)
    out = nc.declare_dram_parameter("out", [C, HW], BF16, isOutput=True)
    trace_sim = bool(os.environ.get("TRACE_SIM"))
    with tile.TileContext(nc, trace_sim=trace_sim) as tc:
        with ExitStack() as const_ctx:
            ident, wt_sb = _cattn_consts(const_ctx, tc, wt)
            for _ in range(repeats):
                _cattn_body(tc, q, k, v, ident, wt_sb, out)
    if not no_walrus:
        _split_excess_waits(nc)
    _NC_CACHE[repeats] = nc
    return nc


LAST_RESULT = None


def kernel(q, k, v, w_proj):
    global LAST_RESULT
    q = np.ascontiguousarray(np.asarray(q, dtype=np.float32))
    k = np.ascontiguousarray(np.asarray(k, dtype=np.float32))
    v = np.ascontiguousarray(np.asarray(v, dtype=np.float32))
    w_proj = np.asarray(w_proj, dtype=np.float32)

    nc = _build_nc(1)
    wt = np.ascontiguousarray(w_proj.T).astype(ml_dtypes.bfloat16)
    in_maps = [
        {
            "q": q[b].reshape(NH, C, S),
            "k": k[b].reshape(NH, C, S),
            "v": v[b].reshape(NH, C, S),
            "wt": wt,
        }
        for b in range(B)
    ]
    trace = bool(os.environ.get("BASS_TRACE"))
    res = run_bass_kernel_spmd(nc, in_maps, list(range(B)), trace=trace)
    LAST_RESULT = res
    out = np.stack(
        [np.asarray(res.results[b]["out"]).astype(np.float32) for b in range(B)]
    )
    return out.reshape(B, C, HH, WW)


if __name__ == "__main__":
    rng = np.random.default_rng(0)
    qq = rng.standard_normal((B, C, HH, WW), dtype=np.float32)
    kk = rng.standard_normal((B, C, HH, WW), dtype=np.float32)
    vv = rng.standard_normal((B, C, HH, WW), dtype=np.float32)
    wp = rng.standard_normal((C, C), dtype=np.float32) / np.sqrt(C)
    o = kernel(qq, kk, vv, wp)
    print("out shape:", o.shape, "finite:", np.isfinite(o).all())
